# revision 4
# baseline (speedup 1.0000x reference)
"""Dcls2d depthwise conv (learnable-spacing dilated conv) for Trainium2.

Math: P1/P2 are (1,3,3) -> tap positions shared across all 384 channels.
The 21x21 constructed kernel is a bilinear scatter of the 3x3 weight grid,
so the conv is a sum of <=36 integer-shifted copies of the input, each
scaled by a per-channel coefficient:

    out[n,c,y,x] = bias[c] + sum_j coef[c,j] * in[n,c,y+dy_j,x+dx_j]

Sharding: data-parallel over batch, 32 imgs -> 4 per core on 8 cores.
Shift positions are computed on host from P1/P2 (tiny) and baked into the
compiled kernel's access patterns; per-channel coefficients are a runtime
input tensor.
"""

import time
from contextlib import ExitStack

import numpy as np

import concourse.tile as tile
from concourse import bacc, mybir

F32 = mybir.dt.float32
F32R = mybir.dt.float32r
ALU = mybir.AluOpType

N, C, H, W = 32, 384, 56, 56
NCORES = 8
NPER = N // NCORES  # 4 images per core
K0 = K1 = 3
D0 = D1 = 7
L0 = L1 = 21  # constructed kernel size
PAD = 10
NBLK = C // 128  # 3 channel blocks
HP, WP = H + 2 * PAD, W + 2 * PAD  # 76x76 padded tile
RPC = 7                      # output rows per PSUM chunk
HHALF = H // 2               # PE works in 28-row halves (4 banks each)
NCHUNK = HHALF // RPC        # 4 chunks of 7*56=392 columns per half
CHW = RPC * W                # 392

# how many taps run as exact-fp32 DVE FMAs (the rest go to the
# TensorEngine as float32r diagonal matmuls); tuned on the cost model.
NDVE = 10


def _host_taps(weight, P1, P2):
    """Bilinear scatter on host -> list of ((dy, dx), coef[384]) taps."""
    w = np.asarray(weight, np.float64).reshape(C, K0 * K1)  # Cg == 1
    p1 = np.clip(np.asarray(P1, np.float64).reshape(-1) + L0 // 2, 0.0, L0 - 1.0)
    p2 = np.clip(np.asarray(P2, np.float64).reshape(-1) + L1 // 2, 0.0, L1 - 1.0)
    f1, f2 = np.floor(p1), np.floor(p2)
    r1, r2 = p1 - f1, p2 - f2
    i1, i2 = f1.astype(int), f2.astype(int)
    i1p = np.minimum(i1 + 1, L0 - 1)
    i2p = np.minimum(i2 + 1, L1 - 1)

    acc = {}  # (a, b) -> coef vector (float64)
    for t in range(K0 * K1):
        for a, b, cf in (
            (i1[t], i2[t], (1 - r1[t]) * (1 - r2[t])),
            (i1p[t], i2[t], r1[t] * (1 - r2[t])),
            (i1[t], i2p[t], (1 - r1[t]) * r2[t]),
            (i1p[t], i2p[t], r1[t] * r2[t]),
        ):
            key = (int(a), int(b))
            v = acc.setdefault(key, np.zeros(C, np.float64))
            v += w[:, t] * cf

    taps = [((a - PAD, b - PAD), v) for (a, b), v in sorted(acc.items())]
    return taps


def _build_hybrid(dve_taps, pe_taps, reps=1):
    """Hybrid TensorE+VectorE per-core program.

    dve_taps: list of (dy, dx) done as exact-fp32 scalar_tensor_tensor on DVE
    pe_taps:  list of (dy, dx) done as float32r diagonal matmuls on TensorE,
              accumulated in PSUM (two 28-row halves, 4 banks each)
    Inputs (x pre-padded, x/diags pre-rounded to f32r on host, shipped as
    raw fp32 bits): x (NPER,C,76,76); coefs (C, n_dve);
    diags (NBLK, n_pe, 128, 128); biasb (C, 1).
    """
    n_dve, n_pe = len(dve_taps), len(pe_taps)
    nc = bacc.Bacc("TRN2", target_bir_lowering=False, debug=False,
                   num_devices=NCORES)
    x = nc.dram_tensor("x", (NPER, C, HP, WP), F32R, kind="ExternalInput").ap()
    coefs = nc.dram_tensor("coefs", (C, max(n_dve, 1)), F32,
                           kind="ExternalInput").ap()
    diags = nc.dram_tensor("diags", (NBLK, n_pe, 128, 128), F32R,
                           kind="ExternalInput").ap()
    biasb = nc.dram_tensor("biasb", (C, 1), F32, kind="ExternalInput").ap()
    out = nc.dram_tensor("out", (NPER, C, H, W), F32, kind="ExternalOutput").ap()

    with tile.TileContext(nc) as tc, ExitStack() as ctx:
        cpool = ctx.enter_context(tc.tile_pool(name="const", bufs=NBLK))
        dpool = ctx.enter_context(tc.tile_pool(name="diag", bufs=NBLK * n_pe))
        ppool = ctx.enter_context(tc.tile_pool(name="pad", bufs=3))
        apool = ctx.enter_context(tc.tile_pool(name="acc", bufs=3))
        pspool = ctx.enter_context(tc.tile_pool(name="psum", bufs=8,
                                                space="PSUM"))

        ct, bt = [], []
        for b in range(NBLK):
            c_t = cpool.tile([128, max(n_dve, 1)], F32, tag="coef")
            nc.sync.dma_start(c_t[:], coefs[128 * b:128 * (b + 1), :])
            ct.append(c_t)
            b_t = cpool.tile([128, 1], F32, tag="bias")
            nc.sync.dma_start(b_t[:], biasb[128 * b:128 * (b + 1), :])
            bt.append(b_t)

        dg = {}
        for b in range(NBLK):
            for k in range(n_pe):
                d_t = dpool.tile([128, 128], F32R, tag="diag",
                                 name=f"diag{b}_{k}")
                nc.sync.dma_start(d_t[:], diags[b, k])
                dg[(b, k)] = d_t

        rep_ctx = tc.For_i(0, reps, 1) if reps > 1 else None
        if rep_ctx is not None:
            ctx.enter_context(rep_ctx)
        for i in range(NPER):
            for b in range(NBLK):
                # padded float32r image block, pre-padded+rounded on host
                xp = ppool.tile([128, HP * WP], F32R, tag="xpad")
                xp3 = xp[:].rearrange("c (h w) -> c h w", w=WP)
                nc.sync.dma_start(
                    xp[:],
                    x[i, 128 * b:128 * (b + 1)].rearrange("c h w -> c (h w)"))

                xpf = xp[:].bitcast(F32).rearrange("c (h w) -> c h w", w=WP)

                # --- VectorE: exact fp32 taps into SBUF accumulator ---
                acc = apool.tile([128, H * W], F32)
                a3 = acc[:].rearrange("c (h w) -> c h w", w=W)
                for t, (dy, dx) in enumerate(dve_taps):
                    if t == 0:
                        # full rect: acc = coef * x_shift (borders read zeros)
                        nc.vector.tensor_scalar(
                            a3[:, :, :],
                            xpf[:, PAD + dy:PAD + dy + H, PAD + dx:PAD + dx + W],
                            ct[b][:, 0:1], None, ALU.mult)
                        continue
                    y0, y1 = max(0, -dy), min(H, H - dy)
                    x0, x1 = max(0, -dx), min(W, W - dx)
                    av = a3[:, y0:y1, x0:x1]
                    xv = xpf[:, PAD + y0 + dy:PAD + y1 + dy,
                             PAD + x0 + dx:PAD + x1 + dx]
                    nc.vector.scalar_tensor_tensor(
                        av, xv, ct[b][:, t:t + 1], av, ALU.mult, ALU.add)

                # --- TensorE: per-tap diagonal matmuls in two 28-row
                # halves (4 PSUM banks each, ping-pong) so one half's
                # merges overlap the other half's matmuls ---
                for hh in range(2):
                    pst = [pspool.tile([128, CHW], F32, tag="ps",
                                       name=f"ps{hh}_{cix}")
                           for cix in range(NCHUNK)]
                    r0 = HHALF * hh
                    # Skip (tap, chunk) matmuls whose 7 output rows lie
                    # entirely outside the tap's valid region (all-zero
                    # contribution from the padding); track first/last
                    # contributing tap per chunk for start/stop flags.
                    contrib = [[] for _ in range(NCHUNK)]
                    for k, (dy, dx) in enumerate(pe_taps):
                        yv0, yv1 = max(0, -dy), min(H, H - dy)
                        for cix in range(NCHUNK):
                            c0 = r0 + RPC * cix
                            if c0 + RPC > yv0 and c0 < yv1:
                                contrib[cix].append(k)
                    for cix in range(NCHUNK):
                        if not contrib[cix]:  # keep psum initialized
                            contrib[cix].append(0)
                    for k, (dy, dx) in enumerate(pe_taps):
                        d_t = dg[(b, k)]
                        for cix in range(NCHUNK):
                            if k not in contrib[cix]:
                                continue
                            y = r0 + RPC * cix + PAD + dy
                            rhs = xp3[:, y:y + RPC, PAD + dx:PAD + dx + W]
                            nc.tensor.matmul(pst[cix][:], d_t[:], rhs,
                                             start=(k == contrib[cix][0]),
                                             stop=(k == contrib[cix][-1]))
                    # merge PSUM + acc + bias on DVE
                    for cix in range(NCHUNK):
                        o0 = (r0 + RPC * cix) * W
                        ac = acc[:, o0:o0 + CHW]
                        nc.vector.scalar_tensor_tensor(
                            ac, pst[cix][:], bt[b][:, 0:1], ac, ALU.add, ALU.add)

                nc.sync.dma_start(
                    out[i, 128 * b:128 * (b + 1)].rearrange("c h w -> c (h w)"),
                    acc[:])

    nc.compile()
    return nc


def _dispatch(nc, in_maps, time_iters=0):
    """Run the compiled Bass module on NCORES cores via PJRT (axon path),
    mirroring bass2jax.run_bass_via_pjrt but with optional repeat-timing on
    device-resident inputs. Returns (results_list, per_call_seconds)."""
    import jax
    from jax.sharding import Mesh, PartitionSpec
    from jax.experimental.shard_map import shard_map
    from concourse import bass2jax, mybir as _mybir
    from concourse.bass2jax import _bass_exec_p, install_neuronx_cc_hook

    install_neuronx_cc_hook()
    n_cores = len(in_maps)

    partition_name = (nc.partition_id_tensor.name
                      if nc.partition_id_tensor else None)
    in_names, out_names, out_avals, zero_outs = [], [], [], []
    for alloc in nc.m.functions[0].allocations:
        if not isinstance(alloc, _mybir.MemoryLocationSet):
            continue
        name = alloc.memorylocations[0].name
        if alloc.kind == "ExternalInput":
            if name != partition_name:
                in_names.append(name)
        elif alloc.kind == "ExternalOutput":
            shape = tuple(alloc.tensor_shape)
            dtype = _mybir.dt.np(alloc.dtype)
            out_names.append(name)
            out_avals.append(jax.core.ShapedArray(shape, dtype))
            zero_outs.append(np.zeros(shape, dtype))
    n_params = len(in_names)
    all_names = in_names + out_names
    if partition_name is not None:
        all_names = all_names + [partition_name]

    def _body(*args):
        operands = list(args)
        if partition_name is not None:
            operands.append(bass2jax.partition_id_tensor())
        outs = _bass_exec_p.bind(
            *operands,
            out_avals=tuple(out_avals),
            in_names=tuple(all_names),
            out_names=tuple(out_names),
            lowering_input_output_aliases=(),
            sim_require_finite=True,
            sim_require_nnan=True,
            nc=nc,
        )
        return tuple(outs)

    devices = jax.devices()[:n_cores]
    mesh = Mesh(np.asarray(devices), ("core",))
    n_args = n_params + len(out_names)
    sharded = jax.jit(
        shard_map(_body, mesh=mesh,
                  in_specs=(PartitionSpec("core"),) * n_args,
                  out_specs=(PartitionSpec("core"),) * len(out_names),
                  check_rep=False),
        keep_unused=True,
    )
    concat_in = [
        np.concatenate([np.asarray(m[name]) for m in in_maps], axis=0)
        for name in in_names
    ]
    concat_zero = [
        np.zeros((n_cores * z.shape[0], *z.shape[1:]), z.dtype) for z in zero_outs
    ]
    sharding = jax.sharding.NamedSharding(mesh, PartitionSpec("core"))
    dev_args = [jax.device_put(a, sharding) for a in concat_in + concat_zero]

    out_arrs = jax.block_until_ready(sharded(*dev_args))
    times = []
    for _ in range(time_iters):
        t0 = time.perf_counter()
        jax.block_until_ready(sharded(*dev_args))
        times.append(time.perf_counter() - t0)

    results = [
        {name: np.asarray(out_arrs[i]).reshape(n_cores, *out_avals[i].shape)[c]
         for i, name in enumerate(out_names)}
        for c in range(n_cores)
    ]
    return results, times


def _null_nc():
    """Tiny kernel through the same path — measures per-call dispatch floor."""
    nc = bacc.Bacc("TRN2", target_bir_lowering=False, debug=False,
                   num_devices=NCORES)
    x = nc.dram_tensor("x", (128, 128), F32, kind="ExternalInput").ap()
    out = nc.dram_tensor("out", (128, 128), F32, kind="ExternalOutput").ap()
    with tile.TileContext(nc) as tc, ExitStack() as ctx:
        pool = ctx.enter_context(tc.tile_pool(name="p", bufs=1))
        t = pool.tile([128, 128], F32)
        nc.sync.dma_start(t[:], x[:])
        nc.sync.dma_start(out[:], t[:])
    nc.compile()
    return nc


def _phys_taps(weight, P1, P2):
    """Per physical tap: (i1, i2, r1, r2, i1p, i2p, wvec[384])."""
    w = np.asarray(weight, np.float64).reshape(C, K0 * K1)
    p1 = np.clip(np.asarray(P1, np.float64).reshape(-1) + L0 // 2, 0.0, L0 - 1.0)
    p2 = np.clip(np.asarray(P2, np.float64).reshape(-1) + L1 // 2, 0.0, L1 - 1.0)
    f1, f2 = np.floor(p1), np.floor(p2)
    out = []
    for t in range(K0 * K1):
        out.append((int(f1[t]), int(f2[t]), float(p1[t] - f1[t]),
                    float(p2[t] - f2[t]), int(min(f1[t] + 1, L0 - 1)),
                    int(min(f2[t] + 1, L1 - 1)), w[:, t]))
    return out


def _prep_sep(input, weight, P1, P2, bias, promote=0, xeps=0.02, tol=0.011):
    """Separable decomposition: per tap one DVE x-interp (u = rho*x[,i2] +
    x[,i2+1], rho=(1-r2)/r2) plus <=2 PE y-legs on u with coef w*r2*(1-r1)
    / w*r2*r1. Taps with r2 within xeps of 0/1 run as direct f32r corners;
    corner sets with max|coef| < tol are dropped (error << rel tolerance)."""
    input = _round_f32r(input)
    input = np.pad(input.reshape(N, C, H, W),
                   ((0, 0), (0, 0), (PAD, PAD), (PAD, PAD)))
    taps = _phys_taps(weight, P1, P2)

    eligible = [t for t, (i1, i2, r1, r2, i1p, i2p, wv) in enumerate(taps)
                if i2p == i2 + 1 and xeps < r2 < 1 - xeps]
    eligible.sort(key=lambda t: -np.abs(taps[t][6]).mean())
    direct_ix = set(range(K0 * K1)) - set(eligible)
    direct_ix |= set(eligible[:promote])
    sep_ix = [t for t in eligible[promote:]]

    # direct corners (merged by position, same as _host_taps)
    dacc = {}
    for t in sorted(direct_ix):
        i1, i2, r1, r2, i1p, i2p, wv = taps[t]
        for a, bb, cf in ((i1, i2, (1 - r1) * (1 - r2)),
                          (i1p, i2, r1 * (1 - r2)),
                          (i1, i2p, (1 - r1) * r2),
                          (i1p, i2p, r1 * r2)):
            v = dacc.setdefault((a, bb), np.zeros(C, np.float64))
            v += wv * cf
    dir_taps = [((a - PAD, bb - PAD), v) for (a, bb), v in sorted(dacc.items())
                if np.abs(v).max() >= tol]

    # separable specs + PE leg list
    sep_specs = []   # (i1, i2, rho, nr)
    leg_specs = []   # (s_idx, row_off = i1 + delta)
    leg_coefs = []
    for s, t in enumerate(sorted(sep_ix)):
        i1, i2, r1, r2, i1p, i2p, wv = taps[t]
        rho = (1 - r2) / r2
        nr = min(57, HP - i1)
        sep_specs.append((i1, i2, rho, nr))
        if i1p == i1:  # y-clamped: single merged leg
            legs = [(0, wv * r2)]
        else:
            legs = []
            if abs(1 - r1) > 1e-12:
                legs.append((0, wv * r2 * (1 - r1)))
            if abs(r1) > 1e-12:
                legs.append((1, wv * r2 * r1))
        for dlt, cv in legs:
            leg_specs.append((s, dlt))
            leg_coefs.append(cv)

    n_mats = len(dir_taps) + len(leg_specs)
    diags = np.zeros((NBLK, max(n_mats, 1), 128, 128), np.float32)
    allc = [v for _, v in dir_taps] + leg_coefs
    for b in range(NBLK):
        for k, v in enumerate(allc):
            np.fill_diagonal(diags[b, k],
                             _round_f32r(v.astype(np.float32)[128 * b:128 * (b + 1)]))
    bias_col = np.asarray(bias, np.float32).reshape(C, 1)
    in_maps = [
        {"x": input[i * NPER:(i + 1) * NPER], "diags": diags, "biasb": bias_col}
        for i in range(NCORES)
    ]
    return [p for p, _ in dir_taps], sep_specs, leg_specs, in_maps


def _build_sep(dir_taps, sep_specs, leg_specs, reps=1):
    """Separable kernel: DVE makes u tiles, PE runs direct corners + legs."""
    n_dir, n_sep, n_leg = len(dir_taps), len(sep_specs), len(leg_specs)
    n_mats = n_dir + n_leg
    nc = bacc.Bacc("TRN2", target_bir_lowering=False, debug=False,
                   num_devices=NCORES)
    x = nc.dram_tensor("x", (NPER, C, HP, WP), F32R, kind="ExternalInput").ap()
    diags = nc.dram_tensor("diags", (NBLK, max(n_mats, 1), 128, 128), F32R,
                           kind="ExternalInput").ap()
    biasb = nc.dram_tensor("biasb", (C, 1), F32, kind="ExternalInput").ap()
    out = nc.dram_tensor("out", (NPER, C, H, W), F32, kind="ExternalOutput").ap()

    with tile.TileContext(nc) as tc, ExitStack() as ctx:
        cpool = ctx.enter_context(tc.tile_pool(name="const", bufs=NBLK))
        dpool = ctx.enter_context(tc.tile_pool(name="diag",
                                               bufs=NBLK * max(n_mats, 1)))
        ppool = ctx.enter_context(tc.tile_pool(name="pad", bufs=2))
        upool = ctx.enter_context(tc.tile_pool(name="uu", bufs=7))
        apool = ctx.enter_context(tc.tile_pool(name="acc", bufs=2))
        pspool = ctx.enter_context(tc.tile_pool(name="psum", bufs=8,
                                                space="PSUM"))

        bt = []
        for b in range(NBLK):
            b_t = cpool.tile([128, 1], F32, tag="bias")
            nc.sync.dma_start(b_t[:], biasb[128 * b:128 * (b + 1), :])
            bt.append(b_t)

        dg = {}
        for b in range(NBLK):
            for k in range(n_mats):
                d_t = dpool.tile([128, 128], F32R, tag="diag",
                                 name=f"diag{b}_{k}")
                nc.sync.dma_start(d_t[:], diags[b, k])
                dg[(b, k)] = d_t

        rep_ctx = tc.For_i(0, reps, 1) if reps > 1 else None
        if rep_ctx is not None:
            ctx.enter_context(rep_ctx)
        for i in range(NPER):
            for b in range(NBLK):
                xp = ppool.tile([128, HP * WP], F32R, tag="xpad")
                xp3 = xp[:].rearrange("c (h w) -> c h w", w=WP)
                nc.sync.dma_start(
                    xp[:],
                    x[i, 128 * b:128 * (b + 1)].rearrange("c h w -> c (h w)"))
                xpf = xp[:].bitcast(F32).rearrange("c (h w) -> c h w", w=WP)

                # --- DVE stage 1: x-interp u tiles (rows rel. to i1) ---
                us = []
                for s, (i1, i2, rho, nr) in enumerate(sep_specs):
                    u = upool.tile([128, 57 * W], F32R, tag="uu",
                                   name=f"u{s}")
                    u3 = u[:].rearrange("c (h w) -> c h w", w=W)
                    nc.vector.scalar_tensor_tensor(
                        u3[:, 0:nr, :],
                        xpf[:, i1:i1 + nr, i2:i2 + W],
                        float(rho),
                        xpf[:, i1:i1 + nr, i2 + 1:i2 + 1 + W],
                        ALU.mult, ALU.add)
                    us.append(u[:].rearrange("c (h w) -> c h w", w=W))

                acc = apool.tile([128, H * W], F32)
                for hh in range(2):
                    pst = [pspool.tile([128, CHW], F32, tag="ps",
                                       name=f"ps{hh}_{cix}")
                           for cix in range(NCHUNK)]
                    r0 = HHALF * hh
                    # contributing op ids per chunk (dirs may skip; legs never)
                    contrib = [[] for _ in range(NCHUNK)]
                    for k, (dy, dx) in enumerate(dir_taps):
                        yv0, yv1 = max(0, -dy), min(H, H - dy)
                        for cix in range(NCHUNK):
                            c0 = r0 + RPC * cix
                            if c0 + RPC > yv0 and c0 < yv1:
                                contrib[cix].append(k)
                    for j in range(n_leg):
                        for cix in range(NCHUNK):
                            contrib[cix].append(n_dir + j)
                    for cix in range(NCHUNK):
                        if not contrib[cix]:
                            contrib[cix].append(0)

                    def mm(op_id, cix, rhs):
                        nc.tensor.matmul(pst[cix][:], dg[(b, op_id)][:], rhs,
                                         start=(op_id == contrib[cix][0]),
                                         stop=(op_id == contrib[cix][-1]))

                    for k, (dy, dx) in enumerate(dir_taps):
                        for cix in range(NCHUNK):
                            if k not in contrib[cix]:
                                continue
                            y = r0 + RPC * cix + PAD + dy
                            mm(k, cix,
                               xp3[:, y:y + RPC, PAD + dx:PAD + dx + W])
                    for j, (s, dlt) in enumerate(leg_specs):
                        for cix in range(NCHUNK):
                            rr = r0 + RPC * cix + dlt
                            mm(n_dir + j, cix, us[s][:, rr:rr + RPC, :])

                    # merge psum + bias into acc on the Scalar engine (ACT),
                    # keeping DVE free for the x-interp ops
                    for cix in range(NCHUNK):
                        o0 = (r0 + RPC * cix) * W
                        nc.scalar.activation(
                            acc[:, o0:o0 + CHW], pst[cix][:],
                            mybir.ActivationFunctionType.Identity,
                            bias=bt[b][:, 0:1])

                nc.sync.dma_start(
                    out[i, 128 * b:128 * (b + 1)].rearrange("c h w -> c (h w)"),
                    acc[:])

    nc.compile()
    return nc


def _run_sep(input, weight, P1, P2, bias, time_iters=0, promote=0):
    dir_taps, sep_specs, leg_specs, in_maps = _prep_sep(
        input, weight, P1, P2, bias, promote=promote)
    nc = _build_sep(dir_taps, sep_specs, leg_specs)
    results, times = _dispatch(nc, in_maps, time_iters=time_iters)
    full = np.concatenate([r["out"] for r in results], axis=0)
    return full, times


def _round_f32r(a):
    """RNE to 11 mantissa bits — the float32r storage format (HW-verified)."""
    b = np.ascontiguousarray(np.asarray(a, np.float32)).view(np.uint32)
    sh = 12
    lsb = (b >> sh) & 1
    r = ((b + np.uint32((1 << (sh - 1)) - 1) + lsb) >> sh) << sh
    return r.view(np.float32)


def _prep(input, weight, P1, P2, bias, n_dve=NDVE):
    """Split taps between DVE (largest |coef|, exact fp32) and PE (f32r)."""
    input = _round_f32r(input)
    input = np.pad(input.reshape(N, C, H, W),
                   ((0, 0), (0, 0), (PAD, PAD), (PAD, PAD)))
    taps = _host_taps(weight, P1, P2)
    assert len(taps) >= 2
    order = np.argsort([-np.abs(v).mean() for _, v in taps])
    n_dve = max(1, min(n_dve, len(taps) - 1))
    dve_ix = sorted(order[:n_dve])
    pe_ix = sorted(order[n_dve:])
    dve_taps = [taps[j][0] for j in dve_ix]
    pe_taps = [taps[j][0] for j in pe_ix]
    n_pe = len(pe_taps)

    if n_dve:
        coefs = np.stack([taps[j][1] for j in dve_ix], axis=1).astype(np.float32)
    else:
        coefs = np.zeros((C, 1), np.float32)
    diags = np.zeros((NBLK, max(n_pe, 1), 128, 128), np.float32)
    for b in range(NBLK):
        for k, j in enumerate(pe_ix):
            v = _round_f32r(taps[j][1].astype(np.float32)[128 * b:128 * (b + 1)])
            np.fill_diagonal(diags[b, k], v)
    bias_col = np.asarray(bias, np.float32).reshape(C, 1)
    in_maps = [
        {"x": input[i * NPER:(i + 1) * NPER], "coefs": coefs, "diags": diags,
         "biasb": bias_col}
        for i in range(NCORES)
    ]
    return dve_taps, pe_taps, in_maps


def _run(input, weight, P1, P2, bias, time_iters=0, n_dve=NDVE):
    dve_taps, pe_taps, in_maps = _prep(input, weight, P1, P2, bias, n_dve=n_dve)
    nc = _build_hybrid(dve_taps, pe_taps)
    results, times = _dispatch(nc, in_maps, time_iters=time_iters)
    full = np.concatenate([r["out"] for r in results], axis=0)
    return full, times


def kernel(input, weight, P1, P2, bias):
    try:
        full, _ = _run_sep(input, weight, P1, P2, bias)
    except Exception:
        full, _ = _run(input, weight, P1, P2, bias)
    return full



# revision 6
# speedup vs baseline: 1.0687x; 1.0687x over previous
"""Dcls2d depthwise conv (learnable-spacing dilated conv) for Trainium2.

Math: P1/P2 are (1,3,3) -> tap positions shared across all 384 channels.
The 21x21 constructed kernel is a bilinear scatter of the 3x3 weight grid,
so the conv is a sum of <=36 integer-shifted copies of the input, each
scaled by a per-channel coefficient:

    out[n,c,y,x] = bias[c] + sum_j coef[c,j] * in[n,c,y+dy_j,x+dx_j]

Sharding: data-parallel over batch, 32 imgs -> 4 per core on 8 cores.
Shift positions are computed on host from P1/P2 (tiny) and baked into the
compiled kernel's access patterns; per-channel coefficients are a runtime
input tensor.

Engine split (v2): 6 interior taps run separably (one DVE x-interp STT +
two TensorE diagonal-matmul y-legs accumulating in PSUM); x-degenerate
taps (r2 within 2% of 0/1) run as direct corner matmuls with
negligible-coefficient corner sets dropped (max|coef| < 0.011, abs error
~0.06 << the 0.115 tolerance); PSUM+bias merges run on the Scalar engine
so the Vector engine only does interpolation. This balances PE ~24us,
DVE ~23us, ACT ~8us per 128-channel block.
"""

import time
from contextlib import ExitStack

import numpy as np

import concourse.tile as tile
from concourse import bacc, mybir

F32 = mybir.dt.float32
F32R = mybir.dt.float32r
ALU = mybir.AluOpType

N, C, H, W = 32, 384, 56, 56
NCORES = 8
NPER = N // NCORES  # 4 images per core
K0 = K1 = 3
D0 = D1 = 7
L0 = L1 = 21  # constructed kernel size
PAD = 10
NBLK = C // 128  # 3 channel blocks
HP, WP = H + 2 * PAD, W + 2 * PAD  # 76x76 padded tile
RPC = 7                      # output rows per PSUM chunk
HHALF = H // 2               # PE works in 28-row halves (4 banks each)
NCHUNK = HHALF // RPC        # 4 chunks of 7*56=392 columns per half
CHW = RPC * W                # 392

# how many taps run as exact-fp32 DVE FMAs (the rest go to the
# TensorEngine as float32r diagonal matmuls); tuned on the cost model.
NDVE = 10


def _host_taps(weight, P1, P2):
    """Bilinear scatter on host -> list of ((dy, dx), coef[384]) taps."""
    w = np.asarray(weight, np.float64).reshape(C, K0 * K1)  # Cg == 1
    p1 = np.clip(np.asarray(P1, np.float64).reshape(-1) + L0 // 2, 0.0, L0 - 1.0)
    p2 = np.clip(np.asarray(P2, np.float64).reshape(-1) + L1 // 2, 0.0, L1 - 1.0)
    f1, f2 = np.floor(p1), np.floor(p2)
    r1, r2 = p1 - f1, p2 - f2
    i1, i2 = f1.astype(int), f2.astype(int)
    i1p = np.minimum(i1 + 1, L0 - 1)
    i2p = np.minimum(i2 + 1, L1 - 1)

    acc = {}  # (a, b) -> coef vector (float64)
    for t in range(K0 * K1):
        for a, b, cf in (
            (i1[t], i2[t], (1 - r1[t]) * (1 - r2[t])),
            (i1p[t], i2[t], r1[t] * (1 - r2[t])),
            (i1[t], i2p[t], (1 - r1[t]) * r2[t]),
            (i1p[t], i2p[t], r1[t] * r2[t]),
        ):
            key = (int(a), int(b))
            v = acc.setdefault(key, np.zeros(C, np.float64))
            v += w[:, t] * cf

    taps = [((a - PAD, b - PAD), v) for (a, b), v in sorted(acc.items())]
    return taps


def _build_hybrid(dve_taps, pe_taps, reps=1):
    """Hybrid TensorE+VectorE per-core program.

    dve_taps: list of (dy, dx) done as exact-fp32 scalar_tensor_tensor on DVE
    pe_taps:  list of (dy, dx) done as float32r diagonal matmuls on TensorE,
              accumulated in PSUM (two 28-row halves, 4 banks each)
    Inputs (x pre-padded, x/diags pre-rounded to f32r on host, shipped as
    raw fp32 bits): x (NPER,C,76,76); coefs (C, n_dve);
    diags (NBLK, n_pe, 128, 128); biasb (C, 1).
    """
    n_dve, n_pe = len(dve_taps), len(pe_taps)
    nc = bacc.Bacc("TRN2", target_bir_lowering=False, debug=False,
                   num_devices=NCORES)
    x = nc.dram_tensor("x", (NPER, C, HP, WP), F32R, kind="ExternalInput").ap()
    coefs = nc.dram_tensor("coefs", (C, max(n_dve, 1)), F32,
                           kind="ExternalInput").ap()
    diags = nc.dram_tensor("diags", (NBLK, n_pe, 128, 128), F32R,
                           kind="ExternalInput").ap()
    biasb = nc.dram_tensor("biasb", (C, 1), F32, kind="ExternalInput").ap()
    out = nc.dram_tensor("out", (NPER, C, H, W), F32, kind="ExternalOutput").ap()

    with tile.TileContext(nc) as tc, ExitStack() as ctx:
        cpool = ctx.enter_context(tc.tile_pool(name="const", bufs=NBLK))
        dpool = ctx.enter_context(tc.tile_pool(name="diag", bufs=NBLK * n_pe))
        ppool = ctx.enter_context(tc.tile_pool(name="pad", bufs=3))
        apool = ctx.enter_context(tc.tile_pool(name="acc", bufs=3))
        pspool = ctx.enter_context(tc.tile_pool(name="psum", bufs=8,
                                                space="PSUM"))

        ct, bt = [], []
        for b in range(NBLK):
            c_t = cpool.tile([128, max(n_dve, 1)], F32, tag="coef")
            nc.sync.dma_start(c_t[:], coefs[128 * b:128 * (b + 1), :])
            ct.append(c_t)
            b_t = cpool.tile([128, 1], F32, tag="bias")
            nc.sync.dma_start(b_t[:], biasb[128 * b:128 * (b + 1), :])
            bt.append(b_t)

        dg = {}
        for b in range(NBLK):
            for k in range(n_pe):
                d_t = dpool.tile([128, 128], F32R, tag="diag",
                                 name=f"diag{b}_{k}")
                nc.sync.dma_start(d_t[:], diags[b, k])
                dg[(b, k)] = d_t

        rep_ctx = tc.For_i(0, reps, 1) if reps > 1 else None
        if rep_ctx is not None:
            ctx.enter_context(rep_ctx)
        for i in range(NPER):
            for b in range(NBLK):
                # padded float32r image block, pre-padded+rounded on host
                xp = ppool.tile([128, HP * WP], F32R, tag="xpad")
                xp3 = xp[:].rearrange("c (h w) -> c h w", w=WP)
                nc.sync.dma_start(
                    xp[:],
                    x[i, 128 * b:128 * (b + 1)].rearrange("c h w -> c (h w)"))

                xpf = xp[:].bitcast(F32).rearrange("c (h w) -> c h w", w=WP)

                # --- VectorE: exact fp32 taps into SBUF accumulator ---
                acc = apool.tile([128, H * W], F32)
                a3 = acc[:].rearrange("c (h w) -> c h w", w=W)
                for t, (dy, dx) in enumerate(dve_taps):
                    if t == 0:
                        # full rect: acc = coef * x_shift (borders read zeros)
                        nc.vector.tensor_scalar(
                            a3[:, :, :],
                            xpf[:, PAD + dy:PAD + dy + H, PAD + dx:PAD + dx + W],
                            ct[b][:, 0:1], None, ALU.mult)
                        continue
                    y0, y1 = max(0, -dy), min(H, H - dy)
                    x0, x1 = max(0, -dx), min(W, W - dx)
                    av = a3[:, y0:y1, x0:x1]
                    xv = xpf[:, PAD + y0 + dy:PAD + y1 + dy,
                             PAD + x0 + dx:PAD + x1 + dx]
                    nc.vector.scalar_tensor_tensor(
                        av, xv, ct[b][:, t:t + 1], av, ALU.mult, ALU.add)

                # --- TensorE: per-tap diagonal matmuls in two 28-row
                # halves (4 PSUM banks each, ping-pong) so one half's
                # merges overlap the other half's matmuls ---
                for hh in range(2):
                    pst = [pspool.tile([128, CHW], F32, tag="ps",
                                       name=f"ps{hh}_{cix}")
                           for cix in range(NCHUNK)]
                    r0 = HHALF * hh
                    # Skip (tap, chunk) matmuls whose 7 output rows lie
                    # entirely outside the tap's valid region (all-zero
                    # contribution from the padding); track first/last
                    # contributing tap per chunk for start/stop flags.
                    contrib = [[] for _ in range(NCHUNK)]
                    for k, (dy, dx) in enumerate(pe_taps):
                        yv0, yv1 = max(0, -dy), min(H, H - dy)
                        for cix in range(NCHUNK):
                            c0 = r0 + RPC * cix
                            if c0 + RPC > yv0 and c0 < yv1:
                                contrib[cix].append(k)
                    for cix in range(NCHUNK):
                        if not contrib[cix]:  # keep psum initialized
                            contrib[cix].append(0)
                    for k, (dy, dx) in enumerate(pe_taps):
                        d_t = dg[(b, k)]
                        for cix in range(NCHUNK):
                            if k not in contrib[cix]:
                                continue
                            y = r0 + RPC * cix + PAD + dy
                            rhs = xp3[:, y:y + RPC, PAD + dx:PAD + dx + W]
                            nc.tensor.matmul(pst[cix][:], d_t[:], rhs,
                                             start=(k == contrib[cix][0]),
                                             stop=(k == contrib[cix][-1]))
                    # merge PSUM + acc + bias on DVE
                    for cix in range(NCHUNK):
                        o0 = (r0 + RPC * cix) * W
                        ac = acc[:, o0:o0 + CHW]
                        nc.vector.scalar_tensor_tensor(
                            ac, pst[cix][:], bt[b][:, 0:1], ac, ALU.add, ALU.add)

                nc.sync.dma_start(
                    out[i, 128 * b:128 * (b + 1)].rearrange("c h w -> c (h w)"),
                    acc[:])

    nc.compile()
    return nc


def _dispatch(nc, in_maps, time_iters=0):
    """Run the compiled Bass module on NCORES cores via PJRT (axon path),
    mirroring bass2jax.run_bass_via_pjrt but with optional repeat-timing on
    device-resident inputs. Returns (results_list, per_call_seconds)."""
    import jax
    from jax.sharding import Mesh, PartitionSpec
    from jax.experimental.shard_map import shard_map
    from concourse import bass2jax, mybir as _mybir
    from concourse.bass2jax import _bass_exec_p, install_neuronx_cc_hook

    install_neuronx_cc_hook()
    n_cores = len(in_maps)

    partition_name = (nc.partition_id_tensor.name
                      if nc.partition_id_tensor else None)
    in_names, out_names, out_avals, zero_outs = [], [], [], []
    for alloc in nc.m.functions[0].allocations:
        if not isinstance(alloc, _mybir.MemoryLocationSet):
            continue
        name = alloc.memorylocations[0].name
        if alloc.kind == "ExternalInput":
            if name != partition_name:
                in_names.append(name)
        elif alloc.kind == "ExternalOutput":
            shape = tuple(alloc.tensor_shape)
            dtype = _mybir.dt.np(alloc.dtype)
            out_names.append(name)
            out_avals.append(jax.core.ShapedArray(shape, dtype))
            zero_outs.append(np.zeros(shape, dtype))
    n_params = len(in_names)
    all_names = in_names + out_names
    if partition_name is not None:
        all_names = all_names + [partition_name]

    def _body(*args):
        operands = list(args)
        if partition_name is not None:
            operands.append(bass2jax.partition_id_tensor())
        outs = _bass_exec_p.bind(
            *operands,
            out_avals=tuple(out_avals),
            in_names=tuple(all_names),
            out_names=tuple(out_names),
            lowering_input_output_aliases=(),
            sim_require_finite=True,
            sim_require_nnan=True,
            nc=nc,
        )
        return tuple(outs)

    devices = jax.devices()[:n_cores]
    mesh = Mesh(np.asarray(devices), ("core",))
    n_args = n_params + len(out_names)
    sharded = jax.jit(
        shard_map(_body, mesh=mesh,
                  in_specs=(PartitionSpec("core"),) * n_args,
                  out_specs=(PartitionSpec("core"),) * len(out_names),
                  check_rep=False),
        keep_unused=True,
    )
    concat_in = [
        np.concatenate([np.asarray(m[name]) for m in in_maps], axis=0)
        for name in in_names
    ]
    concat_zero = [
        np.zeros((n_cores * z.shape[0], *z.shape[1:]), z.dtype) for z in zero_outs
    ]
    sharding = jax.sharding.NamedSharding(mesh, PartitionSpec("core"))
    dev_args = [jax.device_put(a, sharding) for a in concat_in + concat_zero]

    out_arrs = jax.block_until_ready(sharded(*dev_args))
    times = []
    for _ in range(time_iters):
        t0 = time.perf_counter()
        jax.block_until_ready(sharded(*dev_args))
        times.append(time.perf_counter() - t0)

    results = [
        {name: np.asarray(out_arrs[i]).reshape(n_cores, *out_avals[i].shape)[c]
         for i, name in enumerate(out_names)}
        for c in range(n_cores)
    ]
    return results, times


def _null_nc():
    """Tiny kernel through the same path — measures per-call dispatch floor."""
    nc = bacc.Bacc("TRN2", target_bir_lowering=False, debug=False,
                   num_devices=NCORES)
    x = nc.dram_tensor("x", (128, 128), F32, kind="ExternalInput").ap()
    out = nc.dram_tensor("out", (128, 128), F32, kind="ExternalOutput").ap()
    with tile.TileContext(nc) as tc, ExitStack() as ctx:
        pool = ctx.enter_context(tc.tile_pool(name="p", bufs=1))
        t = pool.tile([128, 128], F32)
        nc.sync.dma_start(t[:], x[:])
        nc.sync.dma_start(out[:], t[:])
    nc.compile()
    return nc


def _phys_taps(weight, P1, P2):
    """Per physical tap: (i1, i2, r1, r2, i1p, i2p, wvec[384])."""
    w = np.asarray(weight, np.float64).reshape(C, K0 * K1)
    p1 = np.clip(np.asarray(P1, np.float64).reshape(-1) + L0 // 2, 0.0, L0 - 1.0)
    p2 = np.clip(np.asarray(P2, np.float64).reshape(-1) + L1 // 2, 0.0, L1 - 1.0)
    f1, f2 = np.floor(p1), np.floor(p2)
    out = []
    for t in range(K0 * K1):
        out.append((int(f1[t]), int(f2[t]), float(p1[t] - f1[t]),
                    float(p2[t] - f2[t]), int(min(f1[t] + 1, L0 - 1)),
                    int(min(f2[t] + 1, L1 - 1)), w[:, t]))
    return out


def _prep_sep(input, weight, P1, P2, bias, promote=0, xeps=0.02, tol=0.011):
    """Separable decomposition: per tap one DVE x-interp (u = rho*x[,i2] +
    x[,i2+1], rho=(1-r2)/r2) plus <=2 PE y-legs on u with coef w*r2*(1-r1)
    / w*r2*r1. Taps with r2 within xeps of 0/1 run as direct f32r corners;
    corner sets with max|coef| < tol are dropped (error << rel tolerance)."""
    input = _round_f32r(input)
    input = np.pad(input.reshape(N, C, H, W),
                   ((0, 0), (0, 0), (PAD, PAD), (PAD, PAD)))
    taps = _phys_taps(weight, P1, P2)

    eligible = [t for t, (i1, i2, r1, r2, i1p, i2p, wv) in enumerate(taps)
                if i2p == i2 + 1 and xeps < r2 < 1 - xeps]
    eligible.sort(key=lambda t: -np.abs(taps[t][6]).mean())
    direct_ix = set(range(K0 * K1)) - set(eligible)
    direct_ix |= set(eligible[:promote])
    sep_ix = [t for t in eligible[promote:]]

    # direct corners (merged by position, same as _host_taps)
    dacc = {}
    for t in sorted(direct_ix):
        i1, i2, r1, r2, i1p, i2p, wv = taps[t]
        for a, bb, cf in ((i1, i2, (1 - r1) * (1 - r2)),
                          (i1p, i2, r1 * (1 - r2)),
                          (i1, i2p, (1 - r1) * r2),
                          (i1p, i2p, r1 * r2)):
            v = dacc.setdefault((a, bb), np.zeros(C, np.float64))
            v += wv * cf
    dir_taps = [((a - PAD, bb - PAD), v) for (a, bb), v in sorted(dacc.items())
                if np.abs(v).max() >= tol]

    # separable specs + PE leg list
    sep_specs = []   # (i1, i2, rho, nr)
    leg_specs = []   # (s_idx, row_off = i1 + delta)
    leg_coefs = []
    for s, t in enumerate(sorted(sep_ix)):
        i1, i2, r1, r2, i1p, i2p, wv = taps[t]
        rho = (1 - r2) / r2
        nr = min(57, HP - i1)
        sep_specs.append((i1, i2, rho, nr))
        if i1p == i1:  # y-clamped: single merged leg
            legs = [(0, wv * r2)]
        else:
            legs = []
            if abs(1 - r1) > 1e-12:
                legs.append((0, wv * r2 * (1 - r1)))
            if abs(r1) > 1e-12:
                legs.append((1, wv * r2 * r1))
        for dlt, cv in legs:
            leg_specs.append((s, dlt))
            leg_coefs.append(cv)

    n_mats = len(dir_taps) + len(leg_specs)
    diags = np.zeros((NBLK, max(n_mats, 1), 128, 128), np.float32)
    allc = [v for _, v in dir_taps] + leg_coefs
    for b in range(NBLK):
        for k, v in enumerate(allc):
            np.fill_diagonal(diags[b, k],
                             _round_f32r(v.astype(np.float32)[128 * b:128 * (b + 1)]))
    bias_col = np.asarray(bias, np.float32).reshape(C, 1)
    in_maps = [
        {"x": input[i * NPER:(i + 1) * NPER], "diags": diags, "biasb": bias_col}
        for i in range(NCORES)
    ]
    return [p for p, _ in dir_taps], sep_specs, leg_specs, in_maps


def _build_sep(dir_taps, sep_specs, leg_specs, reps=1):
    """Separable kernel: DVE makes u tiles, PE runs direct corners + legs."""
    n_dir, n_sep, n_leg = len(dir_taps), len(sep_specs), len(leg_specs)
    n_mats = n_dir + n_leg
    nc = bacc.Bacc("TRN2", target_bir_lowering=False, debug=False,
                   num_devices=NCORES)
    x = nc.dram_tensor("x", (NPER, C, HP, WP), F32R, kind="ExternalInput").ap()
    diags = nc.dram_tensor("diags", (NBLK, max(n_mats, 1), 128, 128), F32R,
                           kind="ExternalInput").ap()
    biasb = nc.dram_tensor("biasb", (C, 1), F32, kind="ExternalInput").ap()
    out = nc.dram_tensor("out", (NPER, C, H, W), F32, kind="ExternalOutput").ap()

    with tile.TileContext(nc) as tc, ExitStack() as ctx:
        cpool = ctx.enter_context(tc.tile_pool(name="const", bufs=NBLK))
        dpool = ctx.enter_context(tc.tile_pool(name="diag",
                                               bufs=NBLK * max(n_mats, 1)))
        ppool = ctx.enter_context(tc.tile_pool(name="pad", bufs=2))
        upool = ctx.enter_context(tc.tile_pool(name="uu", bufs=6))
        apool = ctx.enter_context(tc.tile_pool(name="acc", bufs=3))
        pspool = ctx.enter_context(tc.tile_pool(name="psum", bufs=8,
                                                space="PSUM"))

        bt = []
        for b in range(NBLK):
            b_t = cpool.tile([128, 1], F32, tag="bias")
            nc.sync.dma_start(b_t[:], biasb[128 * b:128 * (b + 1), :])
            bt.append(b_t)

        dg = {}
        for b in range(NBLK):
            for k in range(n_mats):
                d_t = dpool.tile([128, 128], F32R, tag="diag",
                                 name=f"diag{b}_{k}")
                nc.sync.dma_start(d_t[:], diags[b, k])
                dg[(b, k)] = d_t

        rep_ctx = tc.For_i(0, reps, 1) if reps > 1 else None
        if rep_ctx is not None:
            ctx.enter_context(rep_ctx)
        for i in range(NPER):
            for b in range(NBLK):
                xp = ppool.tile([128, HP * WP], F32R, tag="xpad")
                xp3 = xp[:].rearrange("c (h w) -> c h w", w=WP)
                nc.sync.dma_start(
                    xp[:],
                    x[i, 128 * b:128 * (b + 1)].rearrange("c h w -> c (h w)"))
                xpf = xp[:].bitcast(F32).rearrange("c (h w) -> c h w", w=WP)

                # --- DVE stage 1: x-interp u tiles (rows rel. to i1) ---
                us = []
                for s, (i1, i2, rho, nr) in enumerate(sep_specs):
                    u = upool.tile([128, 57 * W], F32R, tag="uu",
                                   name=f"u{s}")
                    u3 = u[:].rearrange("c (h w) -> c h w", w=W)
                    nc.vector.scalar_tensor_tensor(
                        u3[:, 0:nr, :],
                        xpf[:, i1:i1 + nr, i2:i2 + W],
                        float(rho),
                        xpf[:, i1:i1 + nr, i2 + 1:i2 + 1 + W],
                        ALU.mult, ALU.add)
                    us.append(u[:].rearrange("c (h w) -> c h w", w=W))

                acc = apool.tile([128, H * W], F32)
                for hh in range(2):
                    pst = [pspool.tile([128, CHW], F32, tag="ps",
                                       name=f"ps{hh}_{cix}")
                           for cix in range(NCHUNK)]
                    r0 = HHALF * hh
                    # contributing op ids per chunk (dirs may skip; legs never)
                    contrib = [[] for _ in range(NCHUNK)]
                    for k, (dy, dx) in enumerate(dir_taps):
                        yv0, yv1 = max(0, -dy), min(H, H - dy)
                        for cix in range(NCHUNK):
                            c0 = r0 + RPC * cix
                            if c0 + RPC > yv0 and c0 < yv1:
                                contrib[cix].append(k)
                    for j in range(n_leg):
                        for cix in range(NCHUNK):
                            contrib[cix].append(n_dir + j)
                    for cix in range(NCHUNK):
                        if not contrib[cix]:
                            contrib[cix].append(0)

                    def mm(op_id, cix, rhs):
                        nc.tensor.matmul(pst[cix][:], dg[(b, op_id)][:], rhs,
                                         start=(op_id == contrib[cix][0]),
                                         stop=(op_id == contrib[cix][-1]))

                    for k, (dy, dx) in enumerate(dir_taps):
                        for cix in range(NCHUNK):
                            if k not in contrib[cix]:
                                continue
                            y = r0 + RPC * cix + PAD + dy
                            mm(k, cix,
                               xp3[:, y:y + RPC, PAD + dx:PAD + dx + W])
                    for j, (s, dlt) in enumerate(leg_specs):
                        for cix in range(NCHUNK):
                            rr = r0 + RPC * cix + dlt
                            mm(n_dir + j, cix, us[s][:, rr:rr + RPC, :])

                    # merge psum + bias into acc on the Scalar engine (ACT),
                    # keeping DVE free for the x-interp ops
                    for cix in range(NCHUNK):
                        o0 = (r0 + RPC * cix) * W
                        nc.scalar.activation(
                            acc[:, o0:o0 + CHW], pst[cix][:],
                            mybir.ActivationFunctionType.Identity,
                            bias=bt[b][:, 0:1])

                nc.sync.dma_start(
                    out[i, 128 * b:128 * (b + 1)].rearrange("c h w -> c (h w)"),
                    acc[:])

    nc.compile()
    return nc


def _run_sep(input, weight, P1, P2, bias, time_iters=0, promote=0):
    dir_taps, sep_specs, leg_specs, in_maps = _prep_sep(
        input, weight, P1, P2, bias, promote=promote)
    nc = _build_sep(dir_taps, sep_specs, leg_specs)
    results, times = _dispatch(nc, in_maps, time_iters=time_iters)
    full = np.concatenate([r["out"] for r in results], axis=0)
    return full, times


def _round_f32r(a):
    """RNE to 11 mantissa bits — the float32r storage format (HW-verified)."""
    b = np.ascontiguousarray(np.asarray(a, np.float32)).view(np.uint32)
    sh = 12
    lsb = (b >> sh) & 1
    r = ((b + np.uint32((1 << (sh - 1)) - 1) + lsb) >> sh) << sh
    return r.view(np.float32)


def _prep(input, weight, P1, P2, bias, n_dve=NDVE):
    """Split taps between DVE (largest |coef|, exact fp32) and PE (f32r)."""
    input = _round_f32r(input)
    input = np.pad(input.reshape(N, C, H, W),
                   ((0, 0), (0, 0), (PAD, PAD), (PAD, PAD)))
    taps = _host_taps(weight, P1, P2)
    assert len(taps) >= 2
    order = np.argsort([-np.abs(v).mean() for _, v in taps])
    n_dve = max(1, min(n_dve, len(taps) - 1))
    dve_ix = sorted(order[:n_dve])
    pe_ix = sorted(order[n_dve:])
    dve_taps = [taps[j][0] for j in dve_ix]
    pe_taps = [taps[j][0] for j in pe_ix]
    n_pe = len(pe_taps)

    if n_dve:
        coefs = np.stack([taps[j][1] for j in dve_ix], axis=1).astype(np.float32)
    else:
        coefs = np.zeros((C, 1), np.float32)
    diags = np.zeros((NBLK, max(n_pe, 1), 128, 128), np.float32)
    for b in range(NBLK):
        for k, j in enumerate(pe_ix):
            v = _round_f32r(taps[j][1].astype(np.float32)[128 * b:128 * (b + 1)])
            np.fill_diagonal(diags[b, k], v)
    bias_col = np.asarray(bias, np.float32).reshape(C, 1)
    in_maps = [
        {"x": input[i * NPER:(i + 1) * NPER], "coefs": coefs, "diags": diags,
         "biasb": bias_col}
        for i in range(NCORES)
    ]
    return dve_taps, pe_taps, in_maps


def _run(input, weight, P1, P2, bias, time_iters=0, n_dve=NDVE):
    dve_taps, pe_taps, in_maps = _prep(input, weight, P1, P2, bias, n_dve=n_dve)
    nc = _build_hybrid(dve_taps, pe_taps)
    results, times = _dispatch(nc, in_maps, time_iters=time_iters)
    full = np.concatenate([r["out"] for r in results], axis=0)
    return full, times


def kernel(input, weight, P1, P2, bias):
    try:
        full, _ = _run_sep(input, weight, P1, P2, bias)
    except Exception:
        full, _ = _run(input, weight, P1, P2, bias)
    return full

